# revision 1
# baseline (speedup 1.0000x reference)
"""MoE routing kernel for Trainium2 (8 NeuronCores, batch-parallel).

Per batch element b (one NeuronCore each):
    pooled = mean_s x[b]; h = tanh(pooled @ rw1 + rb1)
    logits = h @ rw2 + rb2; probs = softmax(logits)
    top-3 of 4 experts, renormalized: w[e] = probs[e]*(probs[e]>min)/(1-min)
    hid_e = gelu_tanh(x[b] @ zw1[e] + zb1[e]); z_e = hid_e @ zw2[e] + zb2[e]
    out[b] = x[b] + sum_e w[e] * z_e

Only the 3 active experts are computed: the router (on device, fp32)
produces the dropped-expert index; three static "slots" then stream
their expert's weights with register-indexed (runtime-offset) DMAs.
Expert MLPs run in fp32r (11-bit mantissa) at full PE rate; weights are
re-laid-out host-side so each dynamic DMA is a clean row-block slice.
"""
import sys

sys.path.insert(0, "/opt/trn_rl_repo")

import numpy as np

import concourse.bacc as bacc
import concourse.bass as bass
import concourse.mybir as mybir
import concourse.tile as tile
import concourse.bass_isa as bass_isa
from concourse.bass_utils import run_bass_kernel_spmd
from concourse.masks import make_identity

S, D, F, E, H = 512, 1024, 4096, 4, 256
K = 3            # active experts (top-3 of 4)
P = 128
TC = S // P      # 4 token chunks
DC = D // P      # 8 d chunks
FC = F // P      # 32 ff chunks
FE = 8           # ff "eighths" for zw1 streaming
FCE = FC // FE   # 4 ff chunks per eighth
NH = D // 512    # 2 output d halves
F32 = mybir.dt.float32
I32 = mybir.dt.int32
F32R = mybir.dt.float32r
GELU = mybir.ActivationFunctionType.Gelu_apprx_tanh


def build_nc():
    nc = bacc.Bacc("TRN2", target_bir_lowering=False, debug=False)

    x_d = nc.dram_tensor("x", [S, D], F32, kind="ExternalInput")
    rw1_d = nc.dram_tensor("rw1", [D, H], F32, kind="ExternalInput")
    rb1_d = nc.dram_tensor("rb1", [H], F32, kind="ExternalInput")
    rw2_d = nc.dram_tensor("rw2", [H, E], F32, kind="ExternalInput")
    rb2_d = nc.dram_tensor("rb2", [E], F32, kind="ExternalInput")
    zw1_d = nc.dram_tensor("zw1r", [E * P, FC, DC, P], F32R, kind="ExternalInput")
    zb1_d = nc.dram_tensor("zb1r", [E * P, FC], F32, kind="ExternalInput")
    zw2_d = nc.dram_tensor("zw2r", [E * F, D], F32R, kind="ExternalInput")
    zb2_d = nc.dram_tensor("zb2", [E, D], F32, kind="ExternalInput")
    out_d = nc.dram_tensor("out", [S, D], F32, kind="ExternalOutput")

    with tile.TileContext(nc) as tc:
        with (
            tc.tile_pool(name="const", bufs=1) as const,
            tc.tile_pool(name="xb", bufs=1) as xb,
            tc.tile_pool(name="wstream", bufs=1) as wstream,
            tc.tile_pool(name="ps", bufs=8, space="PSUM") as ps,
        ):
            ident = const.tile([P, P], F32, name="ident")
            make_identity(nc, ident)
            onesb = const.tile([P, 1], mybir.dt.bfloat16, name="onesb")
            nc.vector.memset(onesb, 1.0)

            # x as 4 quarter DMAs alternating HWDGE rings
            x_all = xb.tile([P, TC, D], F32, name="x_all")
            xv = x_d.rearrange("(t p) d -> p t d", p=P)
            for t in range(TC):
                eng = nc.sync if t % 2 == 0 else nc.scalar
                eng.dma_start(out=x_all[:, t:t + 1, :], in_=xv[:, t:t + 1, :])
            x_sb = [x_all[:, t, :] for t in range(TC)]

            # small router weights on the scalar ring
            rw1_sb = const.tile([P, DC, H], F32, name="rw1_sb")
            nc.scalar.dma_start(out=rw1_sb, in_=rw1_d.rearrange("(c p) h -> p c h", p=P))
            rb1_row = const.tile([1, H], F32, name="rb1_row")
            nc.scalar.dma_start(out=rb1_row, in_=rb1_d.rearrange("(o h) -> o h", o=1))
            rw2T_sb = const.tile([1, E, H], F32, name="rw2T_sb")
            nc.scalar.dma_start(out=rw2T_sb, in_=rw2_d.rearrange("(o h) e -> o e h", o=1))
            rb2_sb = const.tile([1, E], F32, name="rb2_sb")
            nc.scalar.dma_start(out=rb2_sb, in_=rb2_d.rearrange("(o e) -> o e", o=1))
            zb2_sb = const.tile([1, E, D], F32, name="zb2_sb")
            nc.scalar.dma_start(out=zb2_sb, in_=zb2_d.rearrange("(o e) d -> o e d", o=1))

            # bf16 copy of x for the fast (selection-only) router path
            x_bf = xb.tile([P, TC, D], mybir.dt.bfloat16, name="x_bf")
            for t in range(TC):
                nc.vector.tensor_copy(out=x_bf[:, t, :], in_=x_sb[t])

            # ---------- FAST router (bf16): picks the dropped expert ----------
            # pooled_row[1, D] = sum_t ones.T @ x_bf[t]  (1/S folded in later)
            prow_ps = []
            for nh in range(NH):
                pr = ps.tile([1, 512], F32, name=f"prow{nh}", tag="ps")
                for t in range(TC):
                    nc.tensor.matmul(pr, onesb, x_bf[:, t, nh * 512:(nh + 1) * 512],
                                     start=(t == 0), stop=(t == TC - 1))
                prow_ps.append(pr)
            pooled_row = const.tile([1, D], F32, name="pooled_row")
            for nh in range(NH):
                nc.vector.tensor_copy(out=pooled_row[:, nh * 512:(nh + 1) * 512],
                                      in_=prow_ps[nh])
            pooled_dram = nc.dram_tensor("pooled_bounce", [1, D], F32)
            nc.sync.dma_start(out=pooled_dram[:, :], in_=pooled_row)
            pooled3 = const.tile([P, 1, DC], F32, name="pooled3")
            nc.sync.dma_start(out=pooled3,
                              in_=pooled_dram.rearrange("o (c p) -> p o c", p=P))
            pooled = pooled3[:, 0, :]

            # ---------- transposes t0/t1 (PE busy while DVE runs the chain) ----------
            xT = []
            for dc in range(DC):
                xtd = xb.tile([P, S], F32R, name=f"xT{dc}")
                xT.append(xtd)

            def emit_transposes(trange):
                for t in trange:
                    for dc in range(DC):
                        ptr = ps.tile([P, P], F32, name=f"ptr{t}_{dc}", tag="ps")
                        nc.tensor.transpose(ptr, x_sb[t][:, dc * P:(dc + 1) * P], ident)
                        nc.vector.tensor_copy(out=xT[dc][:, t * P:(t + 1) * P], in_=ptr)

            emit_transposes([0, 1])

            # fast h: hacc[p,h] = sum_dc rw1[p,dc,h]*pooled[p,dc];
            # cross-partition sum on gpsimd, then row-form tanh + logits
            hacc = const.tile([P, H], F32, name="hacc")
            nc.vector.tensor_scalar(out=hacc, in0=rw1_sb[:, 0, :],
                                    scalar1=pooled[:, 0:1], scalar2=None,
                                    op0=mybir.AluOpType.mult)
            for dc in range(1, DC):
                nc.vector.scalar_tensor_tensor(out=hacc, in0=rw1_sb[:, dc, :],
                                               scalar=pooled[:, dc:dc + 1], in1=hacc,
                                               op0=mybir.AluOpType.mult,
                                               op1=mybir.AluOpType.add)
            onesf = const.tile([P, 1], F32, name="onesf")
            nc.vector.memset(onesf, 1.0)
            ph = ps.tile([1, H], F32, name="ph", tag="ps")
            nc.tensor.matmul(ph, onesf, hacc, start=True, stop=True)
            hrow_pre = const.tile([1, H], F32, name="hrow_pre")
            nc.vector.scalar_tensor_tensor(out=hrow_pre, in0=ph,
                                           scalar=1.0 / S,
                                           in1=rb1_row, op0=mybir.AluOpType.mult,
                                           op1=mybir.AluOpType.add)
            h_row = const.tile([1, H], F32, name="h_row")
            nc.scalar.activation(out=h_row, in_=hrow_pre,
                                 func=mybir.ActivationFunctionType.Tanh)

            emit_transposes([2])

            logits = const.tile([1, E], F32, name="logits")
            lscr = const.tile([1, H], F32, name="lscr")
            lsum = const.tile([1, E], F32, name="lsum")
            for e in range(E):
                nc.vector.tensor_mul(lscr, h_row, rw2T_sb[:, e, :])
                nc.vector.tensor_reduce(out=lsum[:, e:e + 1], in_=lscr,
                                        axis=mybir.AxisListType.X,
                                        op=mybir.AluOpType.add)
            nc.vector.tensor_add(logits, lsum, rb2_sb)

            # dropped expert straight from logits (argmin; softmax is monotone)
            lmin = const.tile([1, 1], F32, name="lmin")
            nc.vector.tensor_reduce(out=lmin, in_=logits, axis=mybir.AxisListType.X,
                                    op=mybir.AluOpType.min)
            iota4 = const.tile([1, E], F32, name="iota4")
            for e in range(E):
                nc.vector.memset(iota4[:, e:e + 1], float(e))
            lemask = const.tile([1, E], F32, name="lemask")
            nc.vector.tensor_scalar(out=lemask, in0=logits, scalar1=lmin, scalar2=None,
                                    op0=mybir.AluOpType.is_le)
            emul = const.tile([1, E], F32, name="emul")
            nc.vector.tensor_mul(emul, iota4, lemask)
            dminf = const.tile([1, 1], F32, name="dminf")
            nc.vector.tensor_reduce(out=dminf, in_=emul, axis=mybir.AxisListType.X,
                                    op=mybir.AluOpType.add)
            iota3 = const.tile([1, K], F32, name="iota3")
            for k in range(K):
                nc.vector.memset(iota3[:, k:k + 1], float(k))
            gemask = const.tile([1, K], F32, name="gemask")
            nc.vector.tensor_scalar(out=gemask, in0=iota3, scalar1=dminf, scalar2=None,
                                    op0=mybir.AluOpType.is_ge)
            ekf = const.tile([1, K], F32, name="ekf")
            nc.vector.tensor_add(ekf, iota3, gemask)
            ekP_f = const.tile([1, K], F32, name="ekP_f")
            nc.vector.tensor_scalar(out=ekP_f, in0=ekf, scalar1=float(P), scalar2=None,
                                    op0=mybir.AluOpType.mult)
            ekP_i = const.tile([1, K], I32, name="ekP_i")
            nc.vector.tensor_copy(out=ekP_i, in_=ekP_f)
            ekF_f = const.tile([1, K], F32, name="ekF_f")
            nc.vector.tensor_scalar(out=ekF_f, in0=ekf, scalar1=float(F), scalar2=None,
                                    op0=mybir.AluOpType.mult)
            ekF_i = const.tile([1, K], I32, name="ekF_i")
            nc.vector.tensor_copy(out=ekF_i, in_=ekF_f)

            emit_transposes([3])

            hid = xb.tile([P, FC, S], F32R, name="hid")
            # accumulate in place over x_all (x is dead after the transposes)
            zacc = [x_all[:, t, :] for t in range(TC)]
            wbc3 = const.tile([P, 1, K], F32, name="wbc3")
            wbc = wbc3[:, 0, :]

            # ---------- ACCURATE router chain (for combine weights) ----------
            # emitted as closures, interleaved into slot-0 GEMM1 below so the
            # PE never stalls on it; wc only gates the first STT eviction.
            pooledA = const.tile([P, DC], F32, name="pooledA")
            haccA = const.tile([P, H], F32, name="haccA")
            hrow_preA = const.tile([1, H], F32, name="hrow_preA")
            h_rowA = const.tile([1, H], F32, name="h_rowA")
            logitsA = const.tile([1, E], F32, name="logitsA")
            lsumA_box = [None]

            def emit_accA():
                # accurate pooled from xT (rounded x, free-dim reduce on DVE)
                for dc in range(DC):
                    nc.vector.tensor_reduce(out=pooledA[:, dc:dc + 1],
                                            in_=xT[dc].bitcast(F32),
                                            axis=mybir.AxisListType.X,
                                            op=mybir.AluOpType.add)
                nc.vector.tensor_scalar(out=haccA, in0=rw1_sb[:, 0, :],
                                        scalar1=pooledA[:, 0:1], scalar2=None,
                                        op0=mybir.AluOpType.mult)
                for dc in range(1, DC):
                    nc.vector.scalar_tensor_tensor(out=haccA, in0=rw1_sb[:, dc, :],
                                                   scalar=pooledA[:, dc:dc + 1],
                                                   in1=haccA,
                                                   op0=mybir.AluOpType.mult,
                                                   op1=mybir.AluOpType.add)
                hsumA = const.tile([P, H], F32, name="hsumA")
                nc.gpsimd.partition_all_reduce(hsumA, haccA, channels=P,
                                               reduce_op=bass_isa.ReduceOp.add)
                nc.vector.scalar_tensor_tensor(out=hrow_preA, in0=hsumA[0:1, :],
                                               scalar=1.0 / S, in1=rb1_row,
                                               op0=mybir.AluOpType.mult,
                                               op1=mybir.AluOpType.add)

            def emit_accB():
                nc.scalar.activation(out=h_rowA, in_=hrow_preA,
                                     func=mybir.ActivationFunctionType.Tanh)
                lscrA = const.tile([1, H], F32, name="lscrA")
                lsumA_box[0] = const.tile([1, E], F32, name="lsumA")
                for e in range(E):
                    nc.vector.tensor_mul(lscrA, h_rowA, rw2T_sb[:, e, :])
                    nc.vector.tensor_reduce(out=lsumA_box[0][:, e:e + 1], in_=lscrA,
                                            axis=mybir.AxisListType.X,
                                            op=mybir.AluOpType.add)

            def emit_accC():
                nc.vector.tensor_add(logitsA, lsumA_box[0], rb2_sb)
                mx = const.tile([1, 1], F32, name="mx")
                nc.vector.tensor_reduce(out=mx, in_=logitsA,
                                        axis=mybir.AxisListType.X,
                                        op=mybir.AluOpType.max)
                sh = const.tile([1, E], F32, name="sh")
                nc.vector.tensor_scalar(out=sh, in0=logitsA, scalar1=mx,
                                        scalar2=None, op0=mybir.AluOpType.subtract)
                ex = const.tile([1, E], F32, name="ex")
                nc.scalar.activation(out=ex, in_=sh,
                                     func=mybir.ActivationFunctionType.Exp)
                sm = const.tile([1, 1], F32, name="sm")
                nc.vector.tensor_reduce(out=sm, in_=ex, axis=mybir.AxisListType.X,
                                        op=mybir.AluOpType.add)
                rs = const.tile([1, 1], F32, name="rs")
                nc.vector.reciprocal(out=rs, in_=sm)
                probs = const.tile([1, E], F32, name="probs")
                nc.vector.tensor_scalar(out=probs, in0=ex, scalar1=rs, scalar2=None,
                                        op0=mybir.AluOpType.mult)
                pmin = const.tile([1, 1], F32, name="pmin")
                nc.vector.tensor_reduce(out=pmin, in_=probs,
                                        axis=mybir.AxisListType.X,
                                        op=mybir.AluOpType.min)
                onec = const.tile([1, 1], F32, name="onec")
                nc.vector.memset(onec, 1.0)
                den = const.tile([1, 1], F32, name="den")
                nc.vector.tensor_sub(den, onec, pmin)
                rden = const.tile([1, 1], F32, name="rden")
                nc.vector.reciprocal(out=rden, in_=den)
                gtmask = const.tile([1, E], F32, name="gtmask")
                nc.vector.tensor_scalar(out=gtmask, in0=probs, scalar1=pmin,
                                        scalar2=None, op0=mybir.AluOpType.is_gt)
                wall = const.tile([1, E], F32, name="wall")
                nc.vector.tensor_mul(wall, probs, gtmask)
                w_sb = const.tile([1, E], F32, name="w_sb")
                nc.vector.tensor_scalar(out=w_sb, in0=wall, scalar1=rden,
                                        scalar2=None, op0=mybir.AluOpType.mult)
                wdiff = const.tile([1, K], F32, name="wdiff")
                nc.vector.tensor_sub(wdiff, w_sb[:, 1:E], w_sb[:, 0:K])
                wstep = const.tile([1, K], F32, name="wstep")
                nc.vector.tensor_mul(wstep, wdiff, gemask)
                wc = const.tile([1, K], F32, name="wc")
                nc.vector.tensor_add(wc, w_sb[:, 0:K], wstep)
                nc.gpsimd.partition_broadcast(wbc3[:, 0, :], wc, channels=P)
                # zb2sum and residual init
                zb2sum = const.tile([1, D], F32, name="zb2sum")
                nc.vector.tensor_scalar(out=zb2sum, in0=zb2_sb[:, 0, :],
                                        scalar1=w_sb[:, 0:1], scalar2=None,
                                        op0=mybir.AluOpType.mult)
                for e in range(1, E):
                    nc.vector.scalar_tensor_tensor(out=zb2sum, in0=zb2_sb[:, e, :],
                                                   scalar=w_sb[:, e:e + 1],
                                                   in1=zb2sum,
                                                   op0=mybir.AluOpType.mult,
                                                   op1=mybir.AluOpType.add)
                zb2b3 = const.tile([P, 1, D], F32, name="zb2b3")
                nc.gpsimd.partition_broadcast(zb2b3[:, 0, :], zb2sum, channels=P)
                for t in range(TC):
                    nc.vector.tensor_add(zacc[t], zacc[t], zb2b3[:, 0, :])

            # ---------- 3 expert slots, runtime-indexed weight streams ----------
            for k in range(K):
                rF_sy = nc.sync.alloc_register(f"rF_sy{k}")
                nc.reg_load(rF_sy, ekF_i[:, k:k + 1])
                ekF_s = nc.sync.snap(rF_sy)
                rP_sc = nc.scalar.alloc_register(f"rP_sc{k}")
                nc.reg_load(rP_sc, ekP_i[:, k:k + 1])
                ekP_c = nc.scalar.snap(rP_sc)

                zb1_sb = wstream.tile([P, FC], F32, name=f"zb1_sb{k}", tag="zb1",
                                      bufs=2)
                nc.scalar.dma_start(out=zb1_sb, in_=zb1_d[bass.ds(ekP_c, P), :])

                # ---------- GEMM1 ----------
                PRE = 5
                zw1q = [None] * FC

                def load_zw1(fc, kk=k, ek=ekP_c):
                    t = wstream.tile([P, DC, P], F32R, name=f"zw1q{kk}_{fc}",
                                     tag="zw1q", bufs=PRE + 1)
                    nc.scalar.dma_start(out=t, in_=zw1_d[bass.ds(ek, P), fc, :, :])
                    zw1q[fc] = t

                for fc in range(PRE):
                    load_zw1(fc)
                for fc in range(FC):
                    if fc + PRE < FC:
                        load_zw1(fc + PRE)
                    p1 = ps.tile([P, S], F32, name=f"p1_{k}_{fc}", tag="ps")
                    for dc in range(DC):
                        nc.tensor.matmul(p1, zw1q[fc][:, dc, :], xT[dc],
                                         start=(dc == 0), stop=(dc == DC - 1))
                    nc.scalar.activation(out=hid[:, fc, :], in_=p1, func=GELU,
                                         bias=zb1_sb[:, fc:fc + 1], scale=1.0)
                    if k == 0:
                        if fc == 22:
                            emit_accA()
                        elif fc == 27:
                            emit_accB()
                if k == 0:
                    emit_accC()

                # ---------- GEMM2 ----------
                p2 = []
                for t in range(TC):
                    for nh in range(NH):
                        p2t = ps.tile([P, 512], F32, name=f"p2_{k}_{t}_{nh}",
                                      tag="ps")
                        p2.append(p2t)
                for fc in range(FC):
                    zw2t = wstream.tile([P, D], F32R, name=f"zw2t{k}_{fc}",
                                        tag="zw2t", bufs=6)
                    nc.sync.dma_start(out=zw2t,
                                      in_=zw2_d[bass.ds(ekF_s + fc * P, P), :])
                    for t in range(TC):
                        for nh in range(NH):
                            nc.tensor.matmul(
                                p2[t * NH + nh],
                                hid[:, fc, t * P:(t + 1) * P],
                                zw2t[:, nh * 512:(nh + 1) * 512],
                                start=(fc == 0), stop=(fc == FC - 1))
                for t in range(TC):
                    for nh in range(NH):
                        sl = slice(nh * 512, (nh + 1) * 512)
                        nc.vector.scalar_tensor_tensor(
                            out=zacc[t][:, sl], in0=p2[t * NH + nh],
                            scalar=wbc[:, k:k + 1], in1=zacc[t][:, sl],
                            op0=mybir.AluOpType.mult, op1=mybir.AluOpType.add)
                        if k == K - 1:
                            eng = nc.scalar if (t * NH + nh) % 2 == 0 else nc.sync
                            eng.dma_start(out=out_d[t * P:(t + 1) * P, sl],
                                          in_=zacc[t][:, sl])

    nc.finalize()
    return nc


_NC_CACHE = None


def _get_nc():
    global _NC_CACHE
    if _NC_CACHE is None:
        _NC_CACHE = build_nc()
    return _NC_CACHE


def kernel(x, rw1, rb1, rw2, rb2, zw1, zb1, zw2, zb2, **run_kwargs):
    x = np.ascontiguousarray(np.asarray(x, dtype=np.float32))
    zw1 = np.asarray(zw1, np.float32)
    zw2 = np.asarray(zw2, np.float32)
    zb1 = np.asarray(zb1, np.float32)
    # relayouts matching the kernel's dynamic row-block slicing
    # zw1r[e*P+p, fc, dc, fw] = zw1[e, dc*P+p, fc*P+fw]
    zw1r = np.ascontiguousarray(
        zw1.reshape(E, DC, P, FC, P).transpose(0, 2, 3, 1, 4).reshape(E * P, FC, DC, P))
    zb1r = np.ascontiguousarray(
        zb1.reshape(E, FC, P).transpose(0, 2, 1).reshape(E * P, FC))
    zw2r = np.ascontiguousarray(zw2.reshape(E * F, D))
    shared = {
        "rw1": np.ascontiguousarray(np.asarray(rw1, np.float32)),
        "rb1": np.ascontiguousarray(np.asarray(rb1, np.float32)),
        "rw2": np.ascontiguousarray(np.asarray(rw2, np.float32)),
        "rb2": np.ascontiguousarray(np.asarray(rb2, np.float32)),
        "zw1r": zw1r,
        "zb1r": zb1r,
        "zw2r": zw2r,
        "zb2": np.ascontiguousarray(np.asarray(zb2, np.float32)),
    }
    B = x.shape[0]
    nc = _get_nc()
    in_maps = [dict(shared, x=x[b]) for b in range(B)]
    res = run_bass_kernel_spmd(nc, in_maps, core_ids=list(range(B)), **run_kwargs)
    out = np.stack([res.results[b]["out"] for b in range(B)], axis=0)
    if run_kwargs:
        kernel.last_results = res
    return out


if __name__ == "__main__":
    rng = np.random.default_rng(0)
    inputs = {
        "x": rng.standard_normal((8, S, D)).astype(np.float32),
        "rw1": (rng.standard_normal((D, H)) / np.sqrt(D)).astype(np.float32),
        "rb1": np.zeros(H, np.float32),
        "rw2": (rng.standard_normal((H, E)) / np.sqrt(H)).astype(np.float32),
        "rb2": np.zeros(E, np.float32),
        "zw1": (rng.standard_normal((E, D, F)) / np.sqrt(D)).astype(np.float32),
        "zb1": np.zeros((E, F), np.float32),
        "zw2": (rng.standard_normal((E, F, D)) / np.sqrt(F)).astype(np.float32),
        "zb2": np.zeros((E, D), np.float32),
    }
    out = kernel(**inputs)
    print("out", out.shape, out.dtype, np.abs(out).max())



# revision 2
# speedup vs baseline: 1.0238x; 1.0238x over previous
"""MoE routing kernel for Trainium2 (8 NeuronCores, batch-parallel), v3.

Per batch element b (one NeuronCore each):
    pooled = mean_s x[b]; h = tanh(pooled @ rw1 + rb1)
    logits = h @ rw2 + rb2; probs = softmax(logits)
    top-3 of 4 experts, renormalized; out[b] = x[b] + sum_e w[e] * z_e

v3 design:
  - x passed token-major (bf16, residual) and transposed (bf16 for the
    router reduce; fp8e4m3 pair-layout for GEMM1). All weight layout/dtype
    prep host-side.
  - GEMM1: fp8e4m3 + DoubleRow (4 MMs/fc-step, K=256 each).
  - GEMM2: fc 0..7 of the contraction in fp8+DoubleRow (GELU writes those
    chunks as fp8 directly), fc 8..31 in bf16.
  - Router: pooled via DVE free-dim reduce over xT as tiles arrive, then
    h = pooled @ rw1 as 8 tiny bf16 PE matmuls accumulated in PSUM.
    Dummy PE matmuls warm the HAM clock during the router chain.
  - GEMM2 of expert k drains one quantum per GEMM1 fc-step with a
    half-expert lag; 4 (fc-half, d-half) groups of 4 sequential token
    accumulations keep GEMM2 at 2 PSUM banks. hid ping-pongs per expert.
  - Ring layout: sync = xt-even, zw2 stream; scalar = xt-odd, xt8, router
    weights, zb1, zw1 stream, x (deferred); out alternates.
"""
import sys

sys.path.insert(0, "/opt/trn_rl_repo")

import numpy as np
import ml_dtypes

import concourse.bacc as bacc
import concourse.bass as bass
import concourse.mybir as mybir
import concourse.tile as tile
from concourse.bass_utils import run_bass_kernel_spmd

S, D, F, E, H = 512, 1024, 4096, 4, 256
K = 3            # active experts (top-3 of 4)
P = 128
TC = S // P      # 4 token chunks
DC = D // P      # 8 d chunks
DP = DC // 2     # 4 d chunk-pairs (DoubleRow K=256)
FC = F // P      # 32 ff chunks
FH = FC // 2     # fc half
G2F8 = 16        # fc chunks of GEMM2 contraction done in fp8 (pairs: G2F8/2)
F32 = mybir.dt.float32
BF16 = mybir.dt.bfloat16
FP8 = mybir.dt.float8e4
I32 = mybir.dt.int32
GELU = mybir.ActivationFunctionType.Gelu_apprx_tanh
DR = mybir.MatmulPerfMode.DoubleRow
PRE1 = 6         # zw1 pair-tile prefetch depth (covers 2*PRE1 fc chunks)
NB2 = 10         # zw2 bf16 quad-tile pool size


def build_nc():
    nc = bacc.Bacc("TRN2", target_bir_lowering=False, debug=False)

    x_d = nc.dram_tensor("x", [S, D], BF16, kind="ExternalInput")
    xt_d = nc.dram_tensor("xt", [DC * P, S], BF16, kind="ExternalInput")
    xt8_d = nc.dram_tensor("xt8", [DP * P, 2, S], FP8, kind="ExternalInput")
    rw1_d = nc.dram_tensor("rw1b", [D, H], BF16, kind="ExternalInput")
    rb1_d = nc.dram_tensor("rb1", [H], F32, kind="ExternalInput")
    rw2_d = nc.dram_tensor("rw2", [H, E], F32, kind="ExternalInput")
    rb2_d = nc.dram_tensor("rb2", [E], F32, kind="ExternalInput")
    # zw1r8[e*P+p, fcp, j, dcp, i, fw] = zw1[e, (2*dcp+i)*P+p, (2*fcp+j)*P+fw]
    zw1_d = nc.dram_tensor("zw1r", [E * P, FC // 2, 2, DP, 2, P], FP8,
                           kind="ExternalInput")
    # zb1r[e*P+p, fc] = zb1[e, fc*P+p]
    zb1_d = nc.dram_tensor("zb1r", [E * P, FC], F32, kind="ExternalInput")
    # zw28[(e*2+dh)*P+p, fcp, i, j] = zw2[e, (2*fcp+i)*P+p, dh*512+j], fcp<G2F8/2
    zw28_d = nc.dram_tensor("zw28", [E * 2 * P, G2F8 // 2, 2, 512], FP8,
                            kind="ExternalInput")
    # zw2h[(e*2+dh)*P + p, fc-G2F8, j] = zw2[e, fc*P+p, dh*512+j], fc>=G2F8
    # loaded in quads of 4 fc chunks per DMA
    zw2_d = nc.dram_tensor("zw2h", [E * 2 * P, (FC - G2F8) // 4, 4, 512], BF16,
                           kind="ExternalInput")
    zb2_d = nc.dram_tensor("zb2", [E, D], F32, kind="ExternalInput")
    out_d = nc.dram_tensor("out", [S, D], F32, kind="ExternalOutput")

    with tile.TileContext(nc) as tc:
        with (
            tc.tile_pool(name="const", bufs=1) as const,
            tc.tile_pool(name="xb", bufs=1) as xb,
            tc.tile_pool(name="wstream", bufs=1) as wstream,
            tc.tile_pool(name="ps", bufs=8, space="PSUM") as ps,
        ):
            # xt as one tile, four 256KB DMAs split across both rings
            xTall = xb.tile([P, DC, S], BF16, name="xTall")
            xtv = xt_d.rearrange("(c p) s -> p c s", p=P)
            for q in range(4):
                eng = nc.sync if q % 2 == 0 else nc.scalar
                eng.dma_start(out=xTall[:, 2 * q:2 * q + 2, :],
                              in_=xtv[:, 2 * q:2 * q + 2, :])
            xT8all = xb.tile([P, DP, 2, S], FP8, name="xT8all")
            xt8v = xt8_d.rearrange("(c p) i s -> p c i s", p=P)
            nc.sync.dma_start(out=xT8all, in_=xt8v)
            xT8 = [xT8all[:, dcp, :, :] for dcp in range(DP)]

            # x chunks (residual): one DMA on the sync ring, ungated by the
            # router registers, ahead of the zw2 stream
            x_all = xb.tile([P, TC, D], BF16, name="x_all")
            xv = x_d.rearrange("(t p) d -> p t d", p=P)
            nc.sync.dma_start(out=x_all, in_=xv)

            # router weights (scalar ring)
            rw1_sb = const.tile([P, DC, H], BF16, name="rw1_sb")
            nc.scalar.dma_start(out=rw1_sb, in_=rw1_d.rearrange("(c p) h -> p c h", p=P))
            rb1t_sb = const.tile([P, 2], F32, name="rb1t_sb")
            nc.scalar.dma_start(out=rb1t_sb, in_=rb1_d.rearrange("(i p) -> p i", p=P))
            rw2c_sb = const.tile([P, 2, E], F32, name="rw2c_sb")
            nc.scalar.dma_start(out=rw2c_sb, in_=rw2_d.rearrange("(i p) e -> p i e", p=P))
            rb2_sb = const.tile([1, E], F32, name="rb2_sb")
            nc.scalar.dma_start(out=rb2_sb, in_=rb2_d.rearrange("(o e) -> o e", o=1))
            zb2_sb = const.tile([1, E, D], F32, name="zb2_sb")
            nc.scalar.dma_start(out=zb2_sb, in_=zb2_d.rearrange("(o e) d -> o e d", o=1))

            onesb = const.tile([P, 1], BF16, name="onesb")
            nc.vector.memset(onesb, 1.0)

            # ---------- router: pooled reduce + transposed h on the PE ----------
            # phT[i][p] accumulates h_pre[i*128+p] over dc; logits via 2 more MMs.
            pooled_f = const.tile([P, DC], F32, name="pooled_f")
            pooled_col = const.tile([P, DC], BF16, name="pooled_col")
            phT = [ps.tile([P, 1], F32, name=f"phT{i}", tag="phT", bufs=2)
                   for i in range(2)]
            for dc in range(DC):
                nc.vector.tensor_reduce(out=pooled_f[:, dc:dc + 1], in_=xTall[:, dc, :],
                                        axis=mybir.AxisListType.X,
                                        op=mybir.AluOpType.add)
                nc.vector.tensor_copy(out=pooled_col[:, dc:dc + 1],
                                      in_=pooled_f[:, dc:dc + 1])
                for i in range(2):
                    nc.tensor.matmul(phT[i], rw1_sb[:, dc, i * P:(i + 1) * P],
                                     pooled_col[:, dc:dc + 1],
                                     start=(dc == 0), stop=(dc == DC - 1))
            hpreT = const.tile([P, 2], F32, name="hpreT")
            for i in range(2):
                nc.vector.scalar_tensor_tensor(out=hpreT[:, i:i + 1], in0=phT[i],
                                               scalar=1.0 / S,
                                               in1=rb1t_sb[:, i:i + 1],
                                               op0=mybir.AluOpType.mult,
                                               op1=mybir.AluOpType.add)
            hT = const.tile([P, 2], F32, name="hT")
            nc.scalar.activation(out=hT, in_=hpreT,
                                 func=mybir.ActivationFunctionType.Tanh)
            lg = ps.tile([1, E], F32, name="lg", tag="lg", bufs=1)
            for i in range(2):
                nc.tensor.matmul(lg, hT[:, i:i + 1], rw2c_sb[:, i, :],
                                 start=(i == 0), stop=(i == 1))
            logits = const.tile([1, E], F32, name="logits")
            nc.vector.tensor_add(logits, lg, rb2_sb)

            # ---------- PE warm-up (HAM): bridge the selection/fetch window ----------
            warm = ps.tile([1, 512], F32, name="warm", tag="warm", bufs=1)
            for i in range(24):
                nc.tensor.matmul(warm, onesb, xTall[:, 0, :],
                                 start=(i == 0), stop=(i == 23))

            # ---------- selection: dropped expert = argmin(logits) ----------
            lmin = const.tile([1, 1], F32, name="lmin")
            nc.vector.tensor_reduce(out=lmin, in_=logits, axis=mybir.AxisListType.X,
                                    op=mybir.AluOpType.min)
            iota4 = const.tile([1, E], F32, name="iota4")
            for e in range(E):
                nc.vector.memset(iota4[:, e:e + 1], float(e))
            lemask = const.tile([1, E], F32, name="lemask")
            nc.vector.tensor_scalar(out=lemask, in0=logits, scalar1=lmin, scalar2=None,
                                    op0=mybir.AluOpType.is_le)
            emul = const.tile([1, E], F32, name="emul")
            nc.vector.tensor_mul(emul, iota4, lemask)
            dminf = const.tile([1, 1], F32, name="dminf")
            nc.vector.tensor_reduce(out=dminf, in_=emul, axis=mybir.AxisListType.X,
                                    op=mybir.AluOpType.add)
            iota3 = const.tile([1, K], F32, name="iota3")
            for k in range(K):
                nc.vector.memset(iota3[:, k:k + 1], float(k))
            # ekf[k] = k + (k >= dropped)
            gemask = const.tile([1, K], F32, name="gemask")
            nc.vector.tensor_scalar(out=gemask, in0=iota3, scalar1=dminf, scalar2=None,
                                    op0=mybir.AluOpType.is_ge)
            ekf = const.tile([1, K], F32, name="ekf")
            nc.vector.tensor_add(ekf, iota3, gemask)
            ekP_f = const.tile([1, K], F32, name="ekP_f")
            nc.vector.tensor_scalar(out=ekP_f, in0=ekf, scalar1=float(P), scalar2=None,
                                    op0=mybir.AluOpType.mult)
            ekP_i = const.tile([1, K], I32, name="ekP_i")
            nc.vector.tensor_copy(out=ekP_i, in_=ekP_f)
            ekF2_f = const.tile([1, K], F32, name="ekF2_f")
            nc.vector.tensor_scalar(out=ekF2_f, in0=ekf, scalar1=float(2 * P),
                                    scalar2=None, op0=mybir.AluOpType.mult)
            ekF2_i = const.tile([1, K], I32, name="ekF2_i")
            nc.vector.tensor_copy(out=ekF2_i, in_=ekF2_f)

            ekP_c, ekF2_s = [], []
            for k in range(K):
                rP = nc.sync.alloc_register(f"rP_sy{k}")
                nc.reg_load(rP, ekP_i[:, k:k + 1])
                ekP_c.append(nc.sync.snap(rP))
                rF = nc.sync.alloc_register(f"rF_sy{k}")
                nc.reg_load(rF, ekF2_i[:, k:k + 1])
                ekF2_s.append(nc.sync.snap(rF))

            # ---------- expert pipeline state ----------
            wbc3 = const.tile([P, 1, K], F32, name="wbc3")
            wbc = wbc3[:, 0, :]
            zacc = [xb.tile([P, D], F32, name=f"zacc{t}") for t in range(TC)]
            hid = [xb.tile([P, FC - G2F8, S], BF16, name=f"hid{i}") for i in range(2)]
            hid8 = [xb.tile([P, G2F8, S], FP8, name=f"hid8{i}") for i in range(2)]

            zw1q = {}

            def load_zw1(k, fcp):
                # one tile covers fc chunks 2*fcp, 2*fcp+1
                t = wstream.tile([P, 2, DP, 2, P], FP8, name=f"zw1q{k}_{fcp}",
                                 tag="zw1q", bufs=PRE1 + 2)
                nc.sync.dma_start(out=t, in_=zw1_d[bass.ds(ekP_c[k], P), fcp])
                zw1q[(k, fcp)] = t

            zw2q = {}

            def load_zw2(k, dh, fcq):
                # quad tile: fc chunks G2F8 + 4*fcq .. G2F8 + 4*fcq + 3
                t = wstream.tile([P, 4, 512], BF16, name=f"zw2q{k}_{dh}_{fcq}",
                                 tag="zw2q", bufs=NB2)
                nc.sync.dma_start(
                    out=t, in_=zw2_d[bass.ds(ekF2_s[k] + dh * P, P), fcq])
                zw2q[(k, dh, fcq)] = t

            zw28q = {}

            def load_zw28(k, dh):
                t = wstream.tile([P, G2F8 // 2, 2, 512], FP8, name=f"zw28q{k}_{dh}",
                                 tag="zw28q", bufs=3)
                nc.sync.dma_start(out=t, in_=zw28_d[bass.ds(ekF2_s[k] + dh * P, P)])
                zw28q[(k, dh)] = t

            g2_state = {}

            def emit_g2_quantum(k, dh, fch, t, half):
                key = (k, dh, fch, t)
                if key not in g2_state:
                    g2_state[key] = ps.tile([P, 512], F32,
                                            name=f"p2_{k}_{dh}_{fch}_{t}",
                                            tag="p2", bufs=2)
                p2 = g2_state[key]
                if fch == 0:
                    # fp8 DoubleRow over fc 0..15: half0 = pairs 0..3, half1 = 4..7
                    h8 = hid8[k % 2]
                    w8 = zw28q[(k, dh)]
                    for q in range(4):
                        fcp = half * 4 + q
                        nc.tensor.matmul(p2, h8[:, 2 * fcp:2 * fcp + 2,
                                               t * P:(t + 1) * P],
                                         w8[:, fcp, :, :],
                                         start=(fcp == 0), stop=(fcp == 7),
                                         perf_mode=DR)
                else:
                    # bf16 over 8 fc chunks
                    hk = hid[k % 2]
                    for i in range(8):
                        fc = fch * FH + half * 8 + i          # global fc index
                        j = fc - G2F8                         # index into hid/zw2q
                        nc.tensor.matmul(p2, hk[:, j, t * P:(t + 1) * P],
                                         zw2q[(k, dh, j // 4)][:, j % 4, :],
                                         start=(half == 0 and i == 0),
                                         stop=(half == 1 and i == 7))
                if half == 1:
                    # evict this fch's partial accumulation into zacc (additive)
                    sl = slice(dh * 512, (dh + 1) * 512)
                    nc.vector.scalar_tensor_tensor(
                        out=zacc[t][:, sl], in0=p2, scalar=wbc[:, k:k + 1],
                        in1=zacc[t][:, sl], op0=mybir.AluOpType.mult,
                        op1=mybir.AluOpType.add)
                    if k == K - 1 and fch == 1:
                        eng = nc.scalar if (t + dh) % 2 == 0 else nc.sync
                        eng.dma_start(out=out_d[t * P:(t + 1) * P, sl],
                                      in_=zacc[t][:, sl])

            def g2_quanta():
                for k in range(K):
                    for fch in range(2):
                        for dh in range(2):
                            for t in range(TC):
                                for half in range(2):
                                    yield (k, dh, fch, t, half)

            def zw2_tiles():
                for k in range(K):
                    yield ("f8", k, 0)
                    yield ("f8", k, 1)
                    for dh in range(2):
                        for fcq in range((FC - G2F8) // 4):
                            yield ("bf", k, dh, fcq)

            g2_iter = iter(g2_quanta())
            zw2_iter = iter(zw2_tiles())

            def prefetch_zw2(n):
                for _ in range(n):
                    nxt = next(zw2_iter, None)
                    if nxt is None:
                        return
                    if nxt[0] == "f8":
                        load_zw28(nxt[1], nxt[2])
                    else:
                        load_zw2(nxt[1], nxt[2], nxt[3])

            def drain_g2(n=1):
                for _ in range(n):
                    q = next(g2_iter, None)
                    if q is not None:
                        emit_g2_quantum(*q)

            # prologue prefetches: first zw1 tiles lead the scalar ring so GEMM1
            # can start the moment the register snap resolves
            zb1_sb = [wstream.tile([P, FC], F32, name=f"zb1_sb{k}",
                                   tag="zb1", bufs=K) for k in range(K)]
            load_zw1(0, 0)
            nc.sync.dma_start(out=zb1_sb[0], in_=zb1_d[bass.ds(ekP_c[0], P), :])
            for fcp in range(1, PRE1):
                load_zw1(0, fcp)
            prefetch_zw2(3)
            for k in range(1, K):
                nc.sync.dma_start(out=zb1_sb[k], in_=zb1_d[bass.ds(ekP_c[k], P), :])

            # ---------- combine weights (off critical path) ----------
            mx = const.tile([1, 1], F32, name="mx")
            nc.vector.tensor_reduce(out=mx, in_=logits, axis=mybir.AxisListType.X,
                                    op=mybir.AluOpType.max)
            sh = const.tile([1, E], F32, name="sh")
            nc.vector.tensor_scalar(out=sh, in0=logits, scalar1=mx,
                                    scalar2=None, op0=mybir.AluOpType.subtract)
            ex = const.tile([1, E], F32, name="ex")
            nc.scalar.activation(out=ex, in_=sh,
                                 func=mybir.ActivationFunctionType.Exp)
            sm = const.tile([1, 1], F32, name="sm")
            nc.vector.tensor_reduce(out=sm, in_=ex, axis=mybir.AxisListType.X,
                                    op=mybir.AluOpType.add)
            rs = const.tile([1, 1], F32, name="rs")
            nc.vector.reciprocal(out=rs, in_=sm)
            probs = const.tile([1, E], F32, name="probs")
            nc.vector.tensor_scalar(out=probs, in0=ex, scalar1=rs, scalar2=None,
                                    op0=mybir.AluOpType.mult)
            pmin = const.tile([1, 1], F32, name="pmin")
            nc.vector.tensor_reduce(out=pmin, in_=probs, axis=mybir.AxisListType.X,
                                    op=mybir.AluOpType.min)
            onec = const.tile([1, 1], F32, name="onec")
            nc.vector.memset(onec, 1.0)
            den = const.tile([1, 1], F32, name="den")
            nc.vector.tensor_sub(den, onec, pmin)
            rden = const.tile([1, 1], F32, name="rden")
            nc.vector.reciprocal(out=rden, in_=den)
            gtmask = const.tile([1, E], F32, name="gtmask")
            nc.vector.tensor_scalar(out=gtmask, in0=probs, scalar1=pmin,
                                    scalar2=None, op0=mybir.AluOpType.is_gt)
            wall = const.tile([1, E], F32, name="wall")
            nc.vector.tensor_mul(wall, probs, gtmask)
            w_sb = const.tile([1, E], F32, name="w_sb")
            nc.vector.tensor_scalar(out=w_sb, in0=wall, scalar1=rden,
                                    scalar2=None, op0=mybir.AluOpType.mult)
            wdiff = const.tile([1, K], F32, name="wdiff")
            nc.vector.tensor_sub(wdiff, w_sb[:, 1:E], w_sb[:, 0:K])
            wstep = const.tile([1, K], F32, name="wstep")
            nc.vector.tensor_mul(wstep, wdiff, gemask)
            wc = const.tile([1, K], F32, name="wc")
            nc.vector.tensor_add(wc, w_sb[:, 0:K], wstep)
            nc.gpsimd.partition_broadcast(wbc3[:, 0, :], wc, channels=P)
            zb2sum = const.tile([1, D], F32, name="zb2sum")
            nc.vector.tensor_scalar(out=zb2sum, in0=zb2_sb[:, 0, :],
                                    scalar1=w_sb[:, 0:1], scalar2=None,
                                    op0=mybir.AluOpType.mult)
            for e in range(1, E):
                nc.vector.scalar_tensor_tensor(out=zb2sum, in0=zb2_sb[:, e, :],
                                               scalar=w_sb[:, e:e + 1], in1=zb2sum,
                                               op0=mybir.AluOpType.mult,
                                               op1=mybir.AluOpType.add)
            zb2b3 = const.tile([P, 1, D], F32, name="zb2b3")
            nc.gpsimd.partition_broadcast(zb2b3[:, 0, :], zb2sum, channels=P)
            for t in range(TC):
                nc.vector.scalar_tensor_tensor(out=zacc[t], in0=x_all[:, t, :],
                                               scalar=1.0, in1=zb2b3[:, 0, :],
                                               op0=mybir.AluOpType.mult,
                                               op1=mybir.AluOpType.add)

            # ---------- main loop ----------
            NPAIR = FC // 2
            for k in range(K):
                for fc in range(FC):
                    if fc % 2 == 0:
                        nfcp = fc // 2 + PRE1
                        if nfcp < NPAIR:
                            load_zw1(k, nfcp)
                        elif k + 1 < K:
                            load_zw1(k + 1, nfcp - NPAIR)
                    if fc % 3 == 0:
                        prefetch_zw2(1)
                    p1 = ps.tile([P, S], F32, name=f"p1_{k}_{fc}", tag="p1", bufs=2)
                    w1t = zw1q[(k, fc // 2)]
                    for dcp in range(DP):
                        nc.tensor.matmul(p1, w1t[:, fc % 2, dcp, :, :], xT8[dcp],
                                         start=(dcp == 0), stop=(dcp == DP - 1),
                                         perf_mode=DR)
                    if fc < G2F8:
                        nc.scalar.activation(out=hid8[k % 2][:, fc, :], in_=p1,
                                             func=GELU,
                                             bias=zb1_sb[k][:, fc:fc + 1], scale=1.0)
                    else:
                        nc.scalar.activation(out=hid[k % 2][:, fc - G2F8, :], in_=p1,
                                             func=GELU,
                                             bias=zb1_sb[k][:, fc:fc + 1], scale=1.0)
                    if k > 0 or fc >= 16:
                        drain_g2(1)
            # tail: drain remaining G2 work (expert 2 fch=1 groups)
            drain_g2(16)

    nc.finalize()
    return nc


_NC_CACHE = None


def _get_nc():
    global _NC_CACHE
    if _NC_CACHE is None:
        _NC_CACHE = build_nc()
    return _NC_CACHE


def kernel(x, rw1, rb1, rw2, rb2, zw1, zb1, zw2, zb2, **run_kwargs):
    x = np.asarray(x, dtype=np.float32)
    zw1 = np.asarray(zw1, np.float32)
    zw2 = np.asarray(zw2, np.float32)
    zb1 = np.asarray(zb1, np.float32)
    # zw1r8[e*P+p, fcp, j, dcp, i, fw] = zw1[e, (2*dcp+i)*P+p, (2*fcp+j)*P+fw]
    zw1r = np.ascontiguousarray(
        zw1.reshape(E, DP, 2, P, FC // 2, 2, P).transpose(0, 3, 4, 5, 1, 2, 6)
        .reshape(E * P, FC // 2, 2, DP, 2, P)).astype(ml_dtypes.float8_e4m3)
    zb1r = np.ascontiguousarray(
        zb1.reshape(E, FC, P).transpose(0, 2, 1).reshape(E * P, FC))
    # zw2 split: fc < G2F8 in fp8 pair layout, rest bf16
    zw2r = zw2.reshape(E, FC, P, 2, 512)
    # zw28[(e*2+dh)*P+p, fcp, i, j] = zw2[e, (2*fcp+i)*P+p, dh*512+j]
    zw28 = np.ascontiguousarray(
        zw2r[:, :G2F8].reshape(E, G2F8 // 2, 2, P, 2, 512)
        .transpose(0, 4, 3, 1, 2, 5)
        .reshape(E * 2 * P, G2F8 // 2, 2, 512)).astype(ml_dtypes.float8_e4m3)
    # quads: zw2h[(e*2+dh)*P+p, fcq, i, j] = zw2[e, (G2F8+4*fcq+i)*P+p, dh*512+j]
    zw2h = np.ascontiguousarray(
        zw2r[:, G2F8:].reshape(E, (FC - G2F8) // 4, 4, P, 2, 512)
        .transpose(0, 4, 3, 1, 2, 5)
        .reshape(E * 2 * P, (FC - G2F8) // 4, 4, 512)).astype(ml_dtypes.bfloat16)
    shared = {
        "rw1b": np.asarray(rw1, np.float32).astype(ml_dtypes.bfloat16),
        "rb1": np.ascontiguousarray(np.asarray(rb1, np.float32)),
        "rw2": np.ascontiguousarray(np.asarray(rw2, np.float32)),
        "rb2": np.ascontiguousarray(np.asarray(rb2, np.float32)),
        "zw1r": zw1r,
        "zb1r": zb1r,
        "zw28": zw28,
        "zw2h": zw2h,
        "zb2": np.ascontiguousarray(np.asarray(zb2, np.float32)),
    }
    B = x.shape[0]
    nc = _get_nc()
    in_maps = []
    for b in range(B):
        xb_ = x[b]
        xt = np.ascontiguousarray(xb_.T)            # [D, S] fp32
        m = dict(shared, x=xb_.astype(ml_dtypes.bfloat16),
                 xt=xt.astype(ml_dtypes.bfloat16),
                 xt8=np.ascontiguousarray(
                     xt.reshape(DP, 2, P, S).transpose(0, 2, 1, 3)
                     .reshape(DP * P, 2, S)).astype(ml_dtypes.float8_e4m3))
        in_maps.append(m)
    res = run_bass_kernel_spmd(nc, in_maps, core_ids=list(range(B)), **run_kwargs)
    out = np.stack([res.results[b]["out"] for b in range(B)], axis=0)
    if run_kwargs:
        kernel.last_results = res
    return out


if __name__ == "__main__":
    rng = np.random.default_rng(0)
    inputs = {
        "x": rng.standard_normal((8, S, D)).astype(np.float32),
        "rw1": (rng.standard_normal((D, H)) / np.sqrt(D)).astype(np.float32),
        "rb1": np.zeros(H, np.float32),
        "rw2": (rng.standard_normal((H, E)) / np.sqrt(H)).astype(np.float32),
        "rb2": np.zeros(E, np.float32),
        "zw1": (rng.standard_normal((E, D, F)) / np.sqrt(D)).astype(np.float32),
        "zb1": np.zeros((E, F), np.float32),
        "zw2": (rng.standard_normal((E, F, D)) / np.sqrt(F)).astype(np.float32),
        "zb2": np.zeros((E, D), np.float32),
    }
    out = kernel(**inputs)
    print("out", out.shape, out.dtype, np.abs(out).max())


# revision 3
# speedup vs baseline: 1.0251x; 1.0013x over previous
"""MoE routing kernel for Trainium2 (8 NeuronCores, batch-parallel), v3.

Per batch element b (one NeuronCore each):
    pooled = mean_s x[b]; h = tanh(pooled @ rw1 + rb1)
    logits = h @ rw2 + rb2; probs = softmax(logits)
    top-3 of 4 experts, renormalized; out[b] = x[b] + sum_e w[e] * z_e

v3 design:
  - x passed token-major (bf16, residual) and transposed (bf16 for the
    router reduce; fp8e4m3 pair-layout for GEMM1). All weight layout/dtype
    prep host-side.
  - GEMM1: fp8e4m3 + DoubleRow (4 MMs/fc-step, K=256 each).
  - GEMM2: fc 0..7 of the contraction in fp8+DoubleRow (GELU writes those
    chunks as fp8 directly), fc 8..31 in bf16.
  - Router: pooled via DVE free-dim reduce over xT as tiles arrive, then
    h = pooled @ rw1 as 8 tiny bf16 PE matmuls accumulated in PSUM.
    Dummy PE matmuls warm the HAM clock during the router chain.
  - GEMM2 of expert k drains one quantum per GEMM1 fc-step with a
    half-expert lag; 4 (fc-half, d-half) groups of 4 sequential token
    accumulations keep GEMM2 at 2 PSUM banks. hid ping-pongs per expert.
  - Ring layout: sync = xt-even, zw2 stream; scalar = xt-odd, xt8, router
    weights, zb1, zw1 stream, x (deferred); out alternates.
"""
import sys

sys.path.insert(0, "/opt/trn_rl_repo")

import numpy as np
import ml_dtypes

import concourse.bacc as bacc
import concourse.bass as bass
import concourse.mybir as mybir
import concourse.tile as tile
from concourse.bass_utils import run_bass_kernel_spmd

S, D, F, E, H = 512, 1024, 4096, 4, 256
K = 3            # active experts (top-3 of 4)
P = 128
TC = S // P      # 4 token chunks
DC = D // P      # 8 d chunks
DP = DC // 2     # 4 d chunk-pairs (DoubleRow K=256)
FC = F // P      # 32 ff chunks
FH = FC // 2     # fc half
G2F8 = 16        # fc chunks of GEMM2 contraction done in fp8 (pairs: G2F8/2)
F32 = mybir.dt.float32
BF16 = mybir.dt.bfloat16
FP8 = mybir.dt.float8e4
I32 = mybir.dt.int32
GELU = mybir.ActivationFunctionType.Gelu_apprx_tanh
DR = mybir.MatmulPerfMode.DoubleRow
PRE1 = 8         # zw1 pair-tile prefetch depth (covers 2*PRE1 fc chunks)
NB2 = 10         # zw2 bf16 quad-tile pool size


def build_nc():
    nc = bacc.Bacc("TRN2", target_bir_lowering=False, debug=False)

    x_d = nc.dram_tensor("x", [S, D], BF16, kind="ExternalInput")
    xt_d = nc.dram_tensor("xt", [DC * P, S], BF16, kind="ExternalInput")
    xt8_d = nc.dram_tensor("xt8", [DP * P, 2, S], FP8, kind="ExternalInput")
    rw1_d = nc.dram_tensor("rw1b", [D, H], BF16, kind="ExternalInput")
    rb1_d = nc.dram_tensor("rb1", [H], F32, kind="ExternalInput")
    rw2_d = nc.dram_tensor("rw2", [H, E], F32, kind="ExternalInput")
    rb2_d = nc.dram_tensor("rb2", [E], F32, kind="ExternalInput")
    # zw1r8[e*P+p, fcp, j, dcp, i, fw] = zw1[e, (2*dcp+i)*P+p, (2*fcp+j)*P+fw]
    zw1_d = nc.dram_tensor("zw1r", [E * P, FC // 2, 2, DP, 2, P], FP8,
                           kind="ExternalInput")
    # zb1r[e*P+p, fc] = zb1[e, fc*P+p]
    zb1_d = nc.dram_tensor("zb1r", [E * P, FC], F32, kind="ExternalInput")
    # zw28[(e*2+dh)*P+p, fcp, i, j] = zw2[e, (2*fcp+i)*P+p, dh*512+j], fcp<G2F8/2
    zw28_d = nc.dram_tensor("zw28", [E * 2 * P, G2F8 // 2, 2, 512], FP8,
                            kind="ExternalInput")
    # zw2h[(e*2+dh)*P + p, fc-G2F8, j] = zw2[e, fc*P+p, dh*512+j], fc>=G2F8
    # loaded in quads of 4 fc chunks per DMA
    zw2_d = nc.dram_tensor("zw2h", [E * 2 * P, (FC - G2F8) // 4, 4, 512], BF16,
                           kind="ExternalInput")
    zb2_d = nc.dram_tensor("zb2", [E, D], F32, kind="ExternalInput")
    out_d = nc.dram_tensor("out", [S, D], F32, kind="ExternalOutput")

    with tile.TileContext(nc) as tc:
        with (
            tc.tile_pool(name="const", bufs=1) as const,
            tc.tile_pool(name="xb", bufs=1) as xb,
            tc.tile_pool(name="wstream", bufs=1) as wstream,
            tc.tile_pool(name="ps", bufs=8, space="PSUM") as ps,
        ):
            # xt as one tile, four 256KB DMAs split across both rings
            xTall = xb.tile([P, DC, S], BF16, name="xTall")
            xtv = xt_d.rearrange("(c p) s -> p c s", p=P)
            for q in range(4):
                eng = nc.sync if q % 2 == 0 else nc.scalar
                eng.dma_start(out=xTall[:, 2 * q:2 * q + 2, :],
                              in_=xtv[:, 2 * q:2 * q + 2, :])
            xT8all = xb.tile([P, DP, 2, S], FP8, name="xT8all")
            xt8v = xt8_d.rearrange("(c p) i s -> p c i s", p=P)
            nc.sync.dma_start(out=xT8all, in_=xt8v)
            xT8 = [xT8all[:, dcp, :, :] for dcp in range(DP)]

            # x chunks (residual): one DMA on the sync ring, ungated by the
            # router registers, ahead of the zw2 stream
            x_all = xb.tile([P, TC, D], BF16, name="x_all")
            xv = x_d.rearrange("(t p) d -> p t d", p=P)
            nc.sync.dma_start(out=x_all, in_=xv)

            # router weights (scalar ring)
            rw1_sb = const.tile([P, DC, H], BF16, name="rw1_sb")
            nc.scalar.dma_start(out=rw1_sb, in_=rw1_d.rearrange("(c p) h -> p c h", p=P))
            rb1t_sb = const.tile([P, 2], F32, name="rb1t_sb")
            nc.scalar.dma_start(out=rb1t_sb, in_=rb1_d.rearrange("(i p) -> p i", p=P))
            rw2c_sb = const.tile([P, 2, E], F32, name="rw2c_sb")
            nc.scalar.dma_start(out=rw2c_sb, in_=rw2_d.rearrange("(i p) e -> p i e", p=P))
            rb2_sb = const.tile([1, E], F32, name="rb2_sb")
            nc.scalar.dma_start(out=rb2_sb, in_=rb2_d.rearrange("(o e) -> o e", o=1))
            zb2_sb = const.tile([1, E, D], F32, name="zb2_sb")
            nc.scalar.dma_start(out=zb2_sb, in_=zb2_d.rearrange("(o e) d -> o e d", o=1))

            onesb = const.tile([P, 1], BF16, name="onesb")
            nc.vector.memset(onesb, 1.0)

            # ---------- router: pooled reduce + transposed h on the PE ----------
            # phT[i][p] accumulates h_pre[i*128+p] over dc; logits via 2 more MMs.
            pooled_f = const.tile([P, DC], F32, name="pooled_f")
            pooled_col = const.tile([P, DC], BF16, name="pooled_col")
            phT = [ps.tile([P, 1], F32, name=f"phT{i}", tag="phT", bufs=2)
                   for i in range(2)]
            for dc in range(DC):
                nc.vector.tensor_reduce(out=pooled_f[:, dc:dc + 1], in_=xTall[:, dc, :],
                                        axis=mybir.AxisListType.X,
                                        op=mybir.AluOpType.add)
                nc.vector.tensor_copy(out=pooled_col[:, dc:dc + 1],
                                      in_=pooled_f[:, dc:dc + 1])
                for i in range(2):
                    nc.tensor.matmul(phT[i], rw1_sb[:, dc, i * P:(i + 1) * P],
                                     pooled_col[:, dc:dc + 1],
                                     start=(dc == 0), stop=(dc == DC - 1))
            hpreT = const.tile([P, 2], F32, name="hpreT")
            for i in range(2):
                nc.vector.scalar_tensor_tensor(out=hpreT[:, i:i + 1], in0=phT[i],
                                               scalar=1.0 / S,
                                               in1=rb1t_sb[:, i:i + 1],
                                               op0=mybir.AluOpType.mult,
                                               op1=mybir.AluOpType.add)
            hT = const.tile([P, 2], F32, name="hT")
            nc.scalar.activation(out=hT, in_=hpreT,
                                 func=mybir.ActivationFunctionType.Tanh)
            lg = ps.tile([1, E], F32, name="lg", tag="lg", bufs=1)
            for i in range(2):
                nc.tensor.matmul(lg, hT[:, i:i + 1], rw2c_sb[:, i, :],
                                 start=(i == 0), stop=(i == 1))
            logits = const.tile([1, E], F32, name="logits")
            nc.vector.tensor_add(logits, lg, rb2_sb)

            # ---------- PE warm-up (HAM): bridge the selection/fetch window ----------
            warm = ps.tile([1, 512], F32, name="warm", tag="warm", bufs=1)
            for i in range(24):
                nc.tensor.matmul(warm, onesb, xTall[:, 0, :],
                                 start=(i == 0), stop=(i == 23))

            # ---------- selection: dropped expert = argmin(logits) ----------
            lmin = const.tile([1, 1], F32, name="lmin")
            nc.vector.tensor_reduce(out=lmin, in_=logits, axis=mybir.AxisListType.X,
                                    op=mybir.AluOpType.min)
            iota4 = const.tile([1, E], F32, name="iota4")
            for e in range(E):
                nc.vector.memset(iota4[:, e:e + 1], float(e))
            lemask = const.tile([1, E], F32, name="lemask")
            nc.vector.tensor_scalar(out=lemask, in0=logits, scalar1=lmin, scalar2=None,
                                    op0=mybir.AluOpType.is_le)
            emul = const.tile([1, E], F32, name="emul")
            nc.vector.tensor_mul(emul, iota4, lemask)
            dminf = const.tile([1, 1], F32, name="dminf")
            nc.vector.tensor_reduce(out=dminf, in_=emul, axis=mybir.AxisListType.X,
                                    op=mybir.AluOpType.add)
            iota3 = const.tile([1, K], F32, name="iota3")
            for k in range(K):
                nc.vector.memset(iota3[:, k:k + 1], float(k))
            # ekf[k] = k + (k >= dropped)
            gemask = const.tile([1, K], F32, name="gemask")
            nc.vector.tensor_scalar(out=gemask, in0=iota3, scalar1=dminf, scalar2=None,
                                    op0=mybir.AluOpType.is_ge)
            ekf = const.tile([1, K], F32, name="ekf")
            nc.vector.tensor_add(ekf, iota3, gemask)
            ekP_f = const.tile([1, K], F32, name="ekP_f")
            nc.vector.tensor_scalar(out=ekP_f, in0=ekf, scalar1=float(P), scalar2=None,
                                    op0=mybir.AluOpType.mult)
            ekP_i = const.tile([1, K], I32, name="ekP_i")
            nc.vector.tensor_copy(out=ekP_i, in_=ekP_f)
            ekF2_f = const.tile([1, K], F32, name="ekF2_f")
            nc.vector.tensor_scalar(out=ekF2_f, in0=ekf, scalar1=float(2 * P),
                                    scalar2=None, op0=mybir.AluOpType.mult)
            ekF2_i = const.tile([1, K], I32, name="ekF2_i")
            nc.vector.tensor_copy(out=ekF2_i, in_=ekF2_f)

            ekP_c, ekF2_s = [], []
            for k in range(K):
                rP = nc.sync.alloc_register(f"rP_sy{k}")
                nc.reg_load(rP, ekP_i[:, k:k + 1])
                ekP_c.append(nc.sync.snap(rP))
                rF = nc.sync.alloc_register(f"rF_sy{k}")
                nc.reg_load(rF, ekF2_i[:, k:k + 1])
                ekF2_s.append(nc.sync.snap(rF))

            # ---------- expert pipeline state ----------
            wbc3 = const.tile([P, 1, K], F32, name="wbc3")
            wbc = wbc3[:, 0, :]
            zacc = [xb.tile([P, D], F32, name=f"zacc{t}") for t in range(TC)]
            hid = [xb.tile([P, FC - G2F8, S], BF16, name=f"hid{i}") for i in range(2)]
            hid8 = [xb.tile([P, G2F8, S], FP8, name=f"hid8{i}") for i in range(2)]

            zw1q = {}

            def load_zw1(k, fcp):
                # one tile covers fc chunks 2*fcp, 2*fcp+1
                t = wstream.tile([P, 2, DP, 2, P], FP8, name=f"zw1q{k}_{fcp}",
                                 tag="zw1q", bufs=PRE1 + 2)
                nc.sync.dma_start(out=t, in_=zw1_d[bass.ds(ekP_c[k], P), fcp])
                zw1q[(k, fcp)] = t

            zw2q = {}

            def load_zw2(k, dh, fcq):
                # quad tile: fc chunks G2F8 + 4*fcq .. G2F8 + 4*fcq + 3
                t = wstream.tile([P, 4, 512], BF16, name=f"zw2q{k}_{dh}_{fcq}",
                                 tag="zw2q", bufs=NB2)
                nc.sync.dma_start(
                    out=t, in_=zw2_d[bass.ds(ekF2_s[k] + dh * P, P), fcq])
                zw2q[(k, dh, fcq)] = t

            zw28q = {}

            def load_zw28(k, dh):
                t = wstream.tile([P, G2F8 // 2, 2, 512], FP8, name=f"zw28q{k}_{dh}",
                                 tag="zw28q", bufs=3)
                nc.sync.dma_start(out=t, in_=zw28_d[bass.ds(ekF2_s[k] + dh * P, P)])
                zw28q[(k, dh)] = t

            g2_state = {}

            def emit_g2_quantum(k, dh, fch, t, half):
                key = (k, dh, fch, t)
                if key not in g2_state:
                    g2_state[key] = ps.tile([P, 512], F32,
                                            name=f"p2_{k}_{dh}_{fch}_{t}",
                                            tag="p2", bufs=2)
                p2 = g2_state[key]
                if fch == 0:
                    # fp8 DoubleRow over fc 0..15: half0 = pairs 0..3, half1 = 4..7
                    h8 = hid8[k % 2]
                    w8 = zw28q[(k, dh)]
                    for q in range(4):
                        fcp = half * 4 + q
                        nc.tensor.matmul(p2, h8[:, 2 * fcp:2 * fcp + 2,
                                               t * P:(t + 1) * P],
                                         w8[:, fcp, :, :],
                                         start=(fcp == 0), stop=(fcp == 7),
                                         perf_mode=DR)
                else:
                    # bf16 over 8 fc chunks
                    hk = hid[k % 2]
                    for i in range(8):
                        fc = fch * FH + half * 8 + i          # global fc index
                        j = fc - G2F8                         # index into hid/zw2q
                        nc.tensor.matmul(p2, hk[:, j, t * P:(t + 1) * P],
                                         zw2q[(k, dh, j // 4)][:, j % 4, :],
                                         start=(half == 0 and i == 0),
                                         stop=(half == 1 and i == 7))
                if half == 1:
                    # evict this fch's partial accumulation into zacc (additive)
                    sl = slice(dh * 512, (dh + 1) * 512)
                    nc.vector.scalar_tensor_tensor(
                        out=zacc[t][:, sl], in0=p2, scalar=wbc[:, k:k + 1],
                        in1=zacc[t][:, sl], op0=mybir.AluOpType.mult,
                        op1=mybir.AluOpType.add)
                    if k == K - 1 and fch == 1:
                        eng = nc.scalar if (t + dh) % 2 == 0 else nc.sync
                        eng.dma_start(out=out_d[t * P:(t + 1) * P, sl],
                                      in_=zacc[t][:, sl])

            def g2_quanta():
                for k in range(K):
                    for fch in range(2):
                        for dh in range(2):
                            for t in range(TC):
                                for half in range(2):
                                    yield (k, dh, fch, t, half)

            def zw2_tiles():
                for k in range(K):
                    yield ("f8", k, 0)
                    yield ("f8", k, 1)
                    for dh in range(2):
                        for fcq in range((FC - G2F8) // 4):
                            yield ("bf", k, dh, fcq)

            g2_iter = iter(g2_quanta())
            zw2_iter = iter(zw2_tiles())

            def prefetch_zw2(n):
                for _ in range(n):
                    nxt = next(zw2_iter, None)
                    if nxt is None:
                        return
                    if nxt[0] == "f8":
                        load_zw28(nxt[1], nxt[2])
                    else:
                        load_zw2(nxt[1], nxt[2], nxt[3])

            def drain_g2(n=1):
                for _ in range(n):
                    q = next(g2_iter, None)
                    if q is not None:
                        emit_g2_quantum(*q)

            # prologue prefetches: first zw1 tiles lead the scalar ring so GEMM1
            # can start the moment the register snap resolves
            zb1_sb = [wstream.tile([P, FC], F32, name=f"zb1_sb{k}",
                                   tag="zb1", bufs=K) for k in range(K)]
            load_zw1(0, 0)
            nc.sync.dma_start(out=zb1_sb[0], in_=zb1_d[bass.ds(ekP_c[0], P), :])
            for fcp in range(1, PRE1):
                load_zw1(0, fcp)
            prefetch_zw2(1)
            for k in range(1, K):
                nc.sync.dma_start(out=zb1_sb[k], in_=zb1_d[bass.ds(ekP_c[k], P), :])

            # ---------- combine weights (off critical path) ----------
            mx = const.tile([1, 1], F32, name="mx")
            nc.vector.tensor_reduce(out=mx, in_=logits, axis=mybir.AxisListType.X,
                                    op=mybir.AluOpType.max)
            sh = const.tile([1, E], F32, name="sh")
            nc.vector.tensor_scalar(out=sh, in0=logits, scalar1=mx,
                                    scalar2=None, op0=mybir.AluOpType.subtract)
            ex = const.tile([1, E], F32, name="ex")
            nc.scalar.activation(out=ex, in_=sh,
                                 func=mybir.ActivationFunctionType.Exp)
            sm = const.tile([1, 1], F32, name="sm")
            nc.vector.tensor_reduce(out=sm, in_=ex, axis=mybir.AxisListType.X,
                                    op=mybir.AluOpType.add)
            rs = const.tile([1, 1], F32, name="rs")
            nc.vector.reciprocal(out=rs, in_=sm)
            probs = const.tile([1, E], F32, name="probs")
            nc.vector.tensor_scalar(out=probs, in0=ex, scalar1=rs, scalar2=None,
                                    op0=mybir.AluOpType.mult)
            pmin = const.tile([1, 1], F32, name="pmin")
            nc.vector.tensor_reduce(out=pmin, in_=probs, axis=mybir.AxisListType.X,
                                    op=mybir.AluOpType.min)
            onec = const.tile([1, 1], F32, name="onec")
            nc.vector.memset(onec, 1.0)
            den = const.tile([1, 1], F32, name="den")
            nc.vector.tensor_sub(den, onec, pmin)
            rden = const.tile([1, 1], F32, name="rden")
            nc.vector.reciprocal(out=rden, in_=den)
            gtmask = const.tile([1, E], F32, name="gtmask")
            nc.vector.tensor_scalar(out=gtmask, in0=probs, scalar1=pmin,
                                    scalar2=None, op0=mybir.AluOpType.is_gt)
            wall = const.tile([1, E], F32, name="wall")
            nc.vector.tensor_mul(wall, probs, gtmask)
            w_sb = const.tile([1, E], F32, name="w_sb")
            nc.vector.tensor_scalar(out=w_sb, in0=wall, scalar1=rden,
                                    scalar2=None, op0=mybir.AluOpType.mult)
            wdiff = const.tile([1, K], F32, name="wdiff")
            nc.vector.tensor_sub(wdiff, w_sb[:, 1:E], w_sb[:, 0:K])
            wstep = const.tile([1, K], F32, name="wstep")
            nc.vector.tensor_mul(wstep, wdiff, gemask)
            wc = const.tile([1, K], F32, name="wc")
            nc.vector.tensor_add(wc, w_sb[:, 0:K], wstep)
            nc.gpsimd.partition_broadcast(wbc3[:, 0, :], wc, channels=P)
            zb2sum = const.tile([1, D], F32, name="zb2sum")
            nc.vector.tensor_scalar(out=zb2sum, in0=zb2_sb[:, 0, :],
                                    scalar1=w_sb[:, 0:1], scalar2=None,
                                    op0=mybir.AluOpType.mult)
            for e in range(1, E):
                nc.vector.scalar_tensor_tensor(out=zb2sum, in0=zb2_sb[:, e, :],
                                               scalar=w_sb[:, e:e + 1], in1=zb2sum,
                                               op0=mybir.AluOpType.mult,
                                               op1=mybir.AluOpType.add)
            zb2b3 = const.tile([P, 1, D], F32, name="zb2b3")
            nc.gpsimd.partition_broadcast(zb2b3[:, 0, :], zb2sum, channels=P)
            for t in range(TC):
                nc.vector.scalar_tensor_tensor(out=zacc[t], in0=x_all[:, t, :],
                                               scalar=1.0, in1=zb2b3[:, 0, :],
                                               op0=mybir.AluOpType.mult,
                                               op1=mybir.AluOpType.add)

            # ---------- main loop ----------
            NPAIR = FC // 2
            for k in range(K):
                for fc in range(FC):
                    if fc % 2 == 0:
                        nfcp = fc // 2 + PRE1
                        if nfcp < NPAIR:
                            load_zw1(k, nfcp)
                        elif k + 1 < K:
                            load_zw1(k + 1, nfcp - NPAIR)
                    if fc % 3 == 0:
                        prefetch_zw2(1)
                    p1 = ps.tile([P, S], F32, name=f"p1_{k}_{fc}", tag="p1", bufs=2)
                    w1t = zw1q[(k, fc // 2)]
                    for dcp in range(DP):
                        nc.tensor.matmul(p1, w1t[:, fc % 2, dcp, :, :], xT8[dcp],
                                         start=(dcp == 0), stop=(dcp == DP - 1),
                                         perf_mode=DR)
                    if fc < G2F8:
                        nc.scalar.activation(out=hid8[k % 2][:, fc, :], in_=p1,
                                             func=GELU,
                                             bias=zb1_sb[k][:, fc:fc + 1], scale=1.0)
                    else:
                        nc.scalar.activation(out=hid[k % 2][:, fc - G2F8, :], in_=p1,
                                             func=GELU,
                                             bias=zb1_sb[k][:, fc:fc + 1], scale=1.0)
                    if k > 0 or fc >= 16:
                        drain_g2(1)
            # tail: drain remaining G2 work (expert 2 fch=1 groups)
            drain_g2(16)

    nc.finalize()
    return nc


_NC_CACHE = None


def _get_nc():
    global _NC_CACHE
    if _NC_CACHE is None:
        _NC_CACHE = build_nc()
    return _NC_CACHE


def kernel(x, rw1, rb1, rw2, rb2, zw1, zb1, zw2, zb2, **run_kwargs):
    x = np.asarray(x, dtype=np.float32)
    zw1 = np.asarray(zw1, np.float32)
    zw2 = np.asarray(zw2, np.float32)
    zb1 = np.asarray(zb1, np.float32)
    # zw1r8[e*P+p, fcp, j, dcp, i, fw] = zw1[e, (2*dcp+i)*P+p, (2*fcp+j)*P+fw]
    zw1r = np.ascontiguousarray(
        zw1.reshape(E, DP, 2, P, FC // 2, 2, P).transpose(0, 3, 4, 5, 1, 2, 6)
        .reshape(E * P, FC // 2, 2, DP, 2, P)).astype(ml_dtypes.float8_e4m3)
    zb1r = np.ascontiguousarray(
        zb1.reshape(E, FC, P).transpose(0, 2, 1).reshape(E * P, FC))
    # zw2 split: fc < G2F8 in fp8 pair layout, rest bf16
    zw2r = zw2.reshape(E, FC, P, 2, 512)
    # zw28[(e*2+dh)*P+p, fcp, i, j] = zw2[e, (2*fcp+i)*P+p, dh*512+j]
    zw28 = np.ascontiguousarray(
        zw2r[:, :G2F8].reshape(E, G2F8 // 2, 2, P, 2, 512)
        .transpose(0, 4, 3, 1, 2, 5)
        .reshape(E * 2 * P, G2F8 // 2, 2, 512)).astype(ml_dtypes.float8_e4m3)
    # quads: zw2h[(e*2+dh)*P+p, fcq, i, j] = zw2[e, (G2F8+4*fcq+i)*P+p, dh*512+j]
    zw2h = np.ascontiguousarray(
        zw2r[:, G2F8:].reshape(E, (FC - G2F8) // 4, 4, P, 2, 512)
        .transpose(0, 4, 3, 1, 2, 5)
        .reshape(E * 2 * P, (FC - G2F8) // 4, 4, 512)).astype(ml_dtypes.bfloat16)
    shared = {
        "rw1b": np.asarray(rw1, np.float32).astype(ml_dtypes.bfloat16),
        "rb1": np.ascontiguousarray(np.asarray(rb1, np.float32)),
        "rw2": np.ascontiguousarray(np.asarray(rw2, np.float32)),
        "rb2": np.ascontiguousarray(np.asarray(rb2, np.float32)),
        "zw1r": zw1r,
        "zb1r": zb1r,
        "zw28": zw28,
        "zw2h": zw2h,
        "zb2": np.ascontiguousarray(np.asarray(zb2, np.float32)),
    }
    B = x.shape[0]
    nc = _get_nc()
    in_maps = []
    for b in range(B):
        xb_ = x[b]
        xt = np.ascontiguousarray(xb_.T)            # [D, S] fp32
        m = dict(shared, x=xb_.astype(ml_dtypes.bfloat16),
                 xt=xt.astype(ml_dtypes.bfloat16),
                 xt8=np.ascontiguousarray(
                     xt.reshape(DP, 2, P, S).transpose(0, 2, 1, 3)
                     .reshape(DP * P, 2, S)).astype(ml_dtypes.float8_e4m3))
        in_maps.append(m)
    res = run_bass_kernel_spmd(nc, in_maps, core_ids=list(range(B)), **run_kwargs)
    out = np.stack([res.results[b]["out"] for b in range(B)], axis=0)
    if run_kwargs:
        kernel.last_results = res
    return out


if __name__ == "__main__":
    rng = np.random.default_rng(0)
    inputs = {
        "x": rng.standard_normal((8, S, D)).astype(np.float32),
        "rw1": (rng.standard_normal((D, H)) / np.sqrt(D)).astype(np.float32),
        "rb1": np.zeros(H, np.float32),
        "rw2": (rng.standard_normal((H, E)) / np.sqrt(H)).astype(np.float32),
        "rb2": np.zeros(E, np.float32),
        "zw1": (rng.standard_normal((E, D, F)) / np.sqrt(D)).astype(np.float32),
        "zb1": np.zeros((E, F), np.float32),
        "zw2": (rng.standard_normal((E, F, D)) / np.sqrt(F)).astype(np.float32),
        "zb2": np.zeros((E, D), np.float32),
    }
    out = kernel(**inputs)
    print("out", out.shape, out.dtype, np.abs(out).max())
